# revision 1
# baseline (speedup 1.0000x reference)
"""Trainium2 Bass kernel for nn_DeformAttn (deformable attention with patch-FFT).

Self-contained: hardcodes shapes for x [1,48,128,128], 8 NeuronCores.

Strategy (8 cores, y-band split, 16 rows each):
- Layer-1 deformable 1x1: dense 3x3 hat-function taps on x (|off1|<1).
- Layer-2 deformable depthwise 3x3: push W1 through the channel-shared
  sampling: sample the 49-channel s1x (48 ch + in-image indicator) with dense
  5x5 hat taps per kernel point (|off2|<2), then one PE matmul with the
  host-precomputed Khatri-Rao matrix A[c,(k,i)] = w2[c,k]*W1[c,i] (b1/b2 via
  indicator/ones rows).  Offsets off2 via a precomputed OW matrix on s1x.
- Patch FFT (8x8 circular conv): one K=64 matmul with the 2D real-DFT
  matrix, complex pointwise on DVE, one inverse matmul.  Per-pixel LayerNorm.
- Layer-3 deformable 1x1: dense 3x3 taps on vo; host gathers vo between the
  two NEFFs to provide the 1-row cross-band halo.
"""
import numpy as np
import ml_dtypes
_bf16 = ml_dtypes.bfloat16
from contextlib import ExitStack

import concourse.bacc as bacc
import concourse.mybir as mybir
import concourse.tile as tile
from concourse.bass import AP
from concourse import bass_utils

dt = mybir.dt
F32 = dt.float32
F32R = dt.float32r
BF16 = dt.bfloat16
ALU = mybir.AluOpType
ACTF = mybir.ActivationFunctionType
AX = mybir.AxisListType

H = W = 128
BND = 16          # band rows per core
NYX = 24          # x rows per core (band +/-4)
NYS = 22          # s1x rows per core (band +/-3)
XP = 134          # x-padded width (3+128+3)
NCORES = 8
CB = [0, 128, 256, 314]   # uT K-chunk bases over 442 slots


def _v(t, off, dims):
    """View of tile t: keep its full partition dim, custom free dims."""
    return AP(t.tensor, t.offset + off, [list(t.ap[0])] + [list(d) for d in dims])


def build_kernel_a(fold_ln=False):
    nc = bacc.Bacc("TRN2", target_bir_lowering=False, debug=False)
    xh_d = nc.dram_tensor("xh", [48, NYX, XP], F32R, kind="ExternalInput")
    onesp_d = nc.dram_tensor("onesp", [128, NYS], F32, kind="ExternalInput")
    ident_d = nc.dram_tensor("ident", [128, 128], F32, kind="ExternalInput")
    ow1T_d = nc.dram_tensor("ow1T", [48, 2], F32R, kind="ExternalInput")
    owt_d = nc.dram_tensor("owt", [49, 9, 18], F32R, kind="ExternalInput")
    a4_d = nc.dram_tensor("a4", [128, 4, 288], F32R, kind="ExternalInput")
    tfRe_d = nc.dram_tensor("tfRe", [64, 40], F32R, kind="ExternalInput")
    tfIm_d = nc.dram_tensor("tfIm", [64, 40], F32R, kind="ExternalInput")
    tiRe_d = nc.dram_tensor("tiRe", [40, 64], BF16, kind="ExternalInput")
    tiIm_d = nc.dram_tensor("tiIm", [40, 64], BF16, kind="ExternalInput")
    identr_d = nc.dram_tensor("identr", [128, 128], F32R, kind="ExternalInput")
    tau3_d = nc.dram_tensor("tau3", [128, 3], F32, kind="ExternalInput")
    tau5_d = nc.dram_tensor("tau5", [128, 5], F32, kind="ExternalInput")
    lnw_d = nc.dram_tensor("lnw64", [64, 96], F32, kind="ExternalInput")
    lnb_d = nc.dram_tensor("lnb64", [64, 96], F32, kind="ExternalInput")
    vo_out = nc.dram_tensor("vo_out", [96, BND, 128], F32, kind="ExternalOutput")

    with tile.TileContext(nc) as tc, ExitStack() as top:
        cpool = top.enter_context(tc.tile_pool(name="consts", bufs=1))
        ident = cpool.tile([128, 128], F32)
        identr = cpool.tile([128, 128], F32R)
        ow1T = cpool.tile([48, 2], F32R)
        owt = cpool.tile([49, 9, 18], F32R)
        a4 = cpool.tile([128, 4, 288], F32R)
        tfRe = cpool.tile([64, 40], F32R)
        tfIm = cpool.tile([64, 40], F32R)
        tiRe = cpool.tile([40, 64], BF16)
        tiIm = cpool.tile([40, 64], BF16)
        tau3 = cpool.tile([128, 3], F32)
        tau5 = cpool.tile([128, 5], F32)
        lnw = cpool.tile([64, 96], F32)
        lnb = cpool.tile([64, 96], F32)
        for sb, dr in [(ident, ident_d), (identr, identr_d), (ow1T, ow1T_d),
                       (owt, owt_d), (a4, a4_d), (tfRe, tfRe_d), (tfIm, tfIm_d),
                       (tiRe, tiRe_d), (tiIm, tiIm_d), (tau3, tau3_d),
                       (tau5, tau5_d), (lnw, lnw_d), (lnb, lnb_d)]:
            nc.sync.dma_start(sb[:], dr[:])

        psum = top.enter_context(tc.tile_pool(name="psum", bufs=4, space="PSUM"))
        qpsum = top.enter_context(tc.tile_pool(name="qpsum", bufs=2, space="PSUM"))
        dpsum = top.enter_context(tc.tile_pool(name="dpsum", bufs=1, space="PSUM"))

        pqkv_cm = tc.tile_pool(name="pqkv", bufs=1)
        qvpool = pqkv_cm.__enter__()
        qk = qvpool.tile([64, 32, 192], F32R)
        vv = qvpool.tile([64, 32, 96], F32)
        pu_cm = tc.tile_pool(name="pu", bufs=1)
        pupool = pu_cm.__enter__()
        uT = pupool.tile([128, 4, 32, 64], F32R)

        with tc.tile_pool(name="ps1", bufs=1) as s1pool:
            s1x = s1pool.tile([128, 49, NYS], F32)
            off2pm = s1pool.tile([128, BND, 18], F32)
            s1xb = s1pool.tile([128, 49, NYS], BF16)
            s1p = {(0, 0): s1xb}
            for T in (-3, -2, -1, 1, 2, 3):
                s1p[(T, 0)] = s1pool.tile([128, 49, NYS], BF16,
                                          tag=f"s1p{T}", name=f"s1p{T}")
            for T in (-3, -2, -1, 0, 1, 2, 3):
                s1p[(T, 1)] = s1pool.tile([128, 49, NYS], BF16,
                                          tag=f"s1o{T}", name=f"s1o{T}")
            m2 = s1pool.tile([128, 9, 5, 5, BND], BF16)

            # ======== phase X: x load, transposes, off1, m1, s1x, off2 ========
            with tc.tile_pool(name="px", bufs=1) as p1:
                xcm = p1.tile([48, NYX, XP], F32R)
                for rc in range(3):
                    nc.sync.dma_start(xcm[:, 8 * rc:8 * (rc + 1)],
                                      xh_d[:, 8 * rc:8 * (rc + 1)])
                xT = {t: p1.tile([128, 48, NYX], F32, tag=f"xT{t}",
                                 name=f"xT{t}") for t in (-1, 0, 1)}
                for y in range(NYX):
                    ps = psum.tile([128, 512], F32, tag="ps")
                    nc.tensor.transpose(
                        ps[:128, :48].bitcast(F32R),
                        _v(xcm, y * XP + 3, [[1, 128]]),
                        identr[:48, :48])
                    nc.scalar.copy(_v(xT[0], y, [[NYX, 48]]), ps[:128, :48])
                for t in (-1, 1):
                    nc.gpsimd.memset(xT[t][:], 0.0)
                    if t > 0:
                        nc.sync.dma_start(xT[t][0:127], xT[0][1:128])
                    else:
                        nc.sync.dma_start(xT[t][1:128], xT[0][0:127])
                # off1 on s1x rows (xh rows 1..22) -> off1pm [128, 22, 2]
                off1pm = p1.tile([128, NYS, 2], F32)
                for y in range(NYS):
                    ps = psum.tile([128, 512], F32, tag="ps")
                    nc.tensor.matmul(ps[:128, :2],
                                     _v(xcm, (y + 1) * XP + 3, [[1, 128]]),
                                     ow1T[:], start=True, stop=True)
                    nc.scalar.copy(_v(off1pm, y * 2, [[1, 2]]), ps[:128, :2])
                # m1 [128, 3t, 22y, 3s]
                h1y = p1.tile([128, 3, NYS], F32)
                h1x = p1.tile([128, 3, NYS], F32)
                for hh, off in ((h1y, 0), (h1x, 1)):
                    t0 = p1.tile([128, 3, NYS], F32, tag="h1tmp")
                    nc.vector.tensor_tensor(
                        out=t0[:],
                        in0=_v(off1pm, off, [[0, 3], [2, NYS]]),
                        in1=_v(tau3, 0, [[1, 3], [0, NYS]]),
                        op=ALU.subtract)
                    nc.scalar.activation(t0[:], t0[:], ACTF.Abs)
                    nc.scalar.activation(hh[:], t0[:], ACTF.Relu,
                                         bias=1.0, scale=-1.0)
                m1 = p1.tile([128, 3, NYS, 3], F32)
                for t in range(3):
                    nc.vector.tensor_tensor(
                        out=m1[:, t],
                        in0=_v(h1y, 0, [[1, NYS], [NYS, 3]]),
                        in1=_v(h1x, t * NYS, [[1, NYS], [0, 3]]),
                        op=ALU.mult)
                # s1x = sum_t reduce_s( xT_t * m1_t )
                for i, t in enumerate((0, -1, 1)):
                    tmp = p1.tile([128, 48, NYS, 3], F32, tag="s1tmp",
                                  name="s1tmp")
                    nc.vector.tensor_tensor(
                        out=tmp[:],
                        in0=_v(xT[t], 0, [[NYX, 48], [1, NYS], [1, 3]]),
                        in1=_v(m1, (t + 1) * (NYS * 3),
                               [[0, 48], [3, NYS], [1, 3]]),
                        op=ALU.mult)
                    r1 = p1.tile([128, 48, NYS], F32, tag="r1", name="r1",
                                 bufs=2)
                    nc.vector.tensor_tensor(
                        out=r1[:], in0=_v(tmp, 0, [[3, 48 * NYS]]),
                        in1=_v(tmp, 1, [[3, 48 * NYS]]), op=ALU.add)
                    nc.vector.tensor_tensor(
                        out=r1[:], in0=r1[:],
                        in1=_v(tmp, 2, [[3, 48 * NYS]]), op=ALU.add)
                    if i == 0:
                        nc.vector.tensor_copy(
                            _v(s1x, 0, [[NYS, 48], [1, NYS]]), r1[:])
                    else:
                        nc.vector.tensor_tensor(
                            out=_v(s1x, 0, [[NYS, 48], [1, NYS]]),
                            in0=_v(s1x, 0, [[NYS, 48], [1, NYS]]),
                            in1=r1[:], op=ALU.add)
                nc.sync.dma_start(_v(s1x, 48 * NYS, [[1, NYS]]), onesp_d[:])
                # ---- s1xT (channel-major, x-padded) + off2
                s1xT = p1.tile([49, NYS, XP], F32R)
                nc.vector.memset(_v(s1xT, 0, [[XP, NYS], [1, 3]]).bitcast(F32),
                                 0.0)
                nc.vector.memset(
                    _v(s1xT, 131, [[XP, NYS], [1, 3]]).bitcast(F32), 0.0)
                for y in range(NYS):
                    ps = psum.tile([128, 512], F32, tag="ps")
                    nc.tensor.transpose(ps[:49, :128],
                                        _v(s1x, y, [[NYS, 49]]),
                                        ident[:, :])
                    nc.scalar.copy(_v(s1xT, y * XP + 3, [[1, 128]]),
                                   ps[:49, :128])
                for b in range(BND):
                    ps = psum.tile([128, 512], F32, tag="ps")
                    for ty in range(3):
                        for tx in range(3):
                            t = ty * 3 + tx
                            nc.tensor.matmul(
                                ps[:128, :18],
                                _v(s1xT, (b + 2 + ty) * XP + 2 + tx,
                                   [[1, 128]]),
                                owt[:, t], start=(t == 0), stop=(t == 8))
                    nc.scalar.copy(_v(off2pm, b * 18, [[1, 18]]),
                                   ps[:128, :18])
            # ======== phase S: bf16 shifted copies (even+odd y-parity) ====
            nc.vector.tensor_copy(s1xb[:], s1x[:])
            for T in (-3, -2, -1, 1, 2, 3):
                nc.gpsimd.memset(s1p[(T, 0)][:], 0.0)
                if T > 0:
                    nc.sync.dma_start(s1p[(T, 0)][0:128 - T], s1xb[T:128])
                else:
                    nc.sync.dma_start(s1p[(T, 0)][-T:128], s1xb[0:128 + T])
            for T in (-3, -2, -1, 0, 1, 2, 3):
                nc.gpsimd.memset(s1p[(T, 1)][:], 0.0)
                src_odd = _v(s1xb, 1, [[NYS, 49], [1, NYS - 1]])
                dst_odd = _v(s1p[(T, 1)], 0, [[NYS, 49], [1, NYS - 1]])
                if T > 0:
                    nc.sync.dma_start(
                        AP(dst_odd.tensor, dst_odd.offset,
                           [[NYS * 49, 128 - T]] + [list(d) for d in dst_odd.ap[1:]]),
                        AP(src_odd.tensor, src_odd.offset + T * NYS * 49,
                           [[NYS * 49, 128 - T]] + [list(d) for d in src_odd.ap[1:]]))
                elif T < 0:
                    nc.sync.dma_start(
                        AP(dst_odd.tensor, dst_odd.offset + (-T) * NYS * 49,
                           [[NYS * 49, 128 + T]] + [list(d) for d in dst_odd.ap[1:]]),
                        AP(src_odd.tensor, src_odd.offset,
                           [[NYS * 49, 128 + T]] + [list(d) for d in src_odd.ap[1:]]))
                else:
                    nc.sync.dma_start(dst_odd, src_odd)
            with tc.tile_pool(name="pU", bufs=1) as upool:
                u = upool.tile([128, 442, BND], F32)
                h2y = upool.tile([128, 9, BND, 5], F32)
                h2x = upool.tile([128, 9, BND, 5], F32)
                for hh, off in ((h2y, 0), (h2x, 1)):
                    t0 = upool.tile([128, 9, BND, 5], F32, tag="h2tmp",
                                    name="h2tmp", bufs=2)
                    nc.vector.tensor_tensor(
                        out=t0[:],
                        in0=_v(off2pm, off, [[2, 9], [18, BND], [0, 5]]),
                        in1=_v(tau5, 0, [[0, 9], [0, BND], [1, 5]]),
                        op=ALU.subtract)
                    nc.scalar.activation(t0[:], t0[:], ACTF.Abs)
                    nc.scalar.activation(hh[:], t0[:], ACTF.Relu,
                                         bias=1.0, scale=-1.0)
                for k in range(9):
                    nc.vector.tensor_tensor(
                        out=m2[:, k],
                        in0=_v(h2y, k * BND * 5, [[0, 5], [1, 5], [5, BND]]),
                        in1=_v(h2x, k * BND * 5, [[1, 5], [0, 5], [5, BND]]),
                        op=ALU.mult)
                # u sampling
                nc.vector.memset(_v(u, 441 * BND, [[1, BND]]), 1.0)
                for k in range(9):
                    ki, kj = divmod(k, 3)
                    for st in range(5):
                        T = kj - 1 + st - 2
                        par = ki % 2
                        src = s1p[(T, par)]
                        off0 = ki - par
                        tmp = upool.tile([128, 5, 49, BND], BF16, tag="utmp",
                                         name="utmp", bufs=3)
                        nc.vector.tensor_tensor(
                            out=tmp[:],
                            in0=_v(src, off0, [[1, 5], [NYS, 49], [1, BND]]),
                            in1=_v(m2, k * 25 * BND + st * 5 * BND,
                                   [[BND, 5], [0, 49], [1, BND]]),
                            op=ALU.mult)
                        ab = upool.tile([128, 2, 49, BND], BF16, tag="ab",
                                        name="ab", bufs=2)
                        a01 = upool.tile([128, 49, BND], BF16, tag="a01",
                                         name="a01", bufs=2)
                        with nc.allow_low_precision(
                                reason="tap partial sums; final accum fp32"):
                            nc.vector.tensor_tensor(
                                out=ab[:],
                                in0=_v(tmp, 0, [[2 * 784, 2], [1, 784]]),
                                in1=_v(tmp, 784, [[2 * 784, 2], [1, 784]]),
                                op=ALU.add)
                            nc.vector.tensor_tensor(
                                out=a01[:], in0=_v(ab, 0, [[1, 784]]),
                                in1=_v(ab, 784, [[1, 784]]), op=ALU.add)
                            nc.vector.tensor_tensor(
                                out=a01[:], in0=a01[:],
                                in1=_v(tmp, 4 * 784, [[1, 784]]), op=ALU.add)
                        if st == 0:
                            nc.gpsimd.tensor_copy(
                                _v(u, k * 49 * BND, [[BND, 49], [1, BND]]),
                                a01[:])
                        else:
                            nc.gpsimd.tensor_tensor(
                                out=_v(u, k * 49 * BND, [[BND, 49], [1, BND]]),
                                in0=_v(u, k * 49 * BND, [[BND, 49], [1, BND]]),
                                in1=a01[:], op=ALU.add)
                # uT (transpose u)
                for c in range(4):
                    for y in range(BND):
                        ps = psum.tile([128, 512], F32, tag="ps")
                        nc.tensor.transpose(
                            ps[:128, :128],
                            _v(u, CB[c] * BND + y, [[BND, 128]]),
                            ident[:, :])
                        nc.scalar.copy(
                            _v(uT, c * 2048 + (y // 8) * 1024 + (y % 8) * 8,
                               [[64, 16], [1, 8]]),
                            ps[:128, :128])
        # s1pool closed: s1x family + m2 freed; uT + qkv alive
        # ======== phase Q: qkv matmul ========
        for pi in range(32):
            prow, pcol = divmod(pi, 16)
            qp = qpsum.tile([64, 288], F32, tag="qp")
            for c in range(4):
                lhsT = _v(uT, c * 2048 + pi * 64, [[1, 64]])
                nc.tensor.matmul(qp[:, :], lhsT, a4[:, c], start=(c == 0),
                                 stop=(c == 3))
            nc.scalar.copy(_v(qk, pi * 192, [[1, 192]]), qp[:, 0:192])
            nc.scalar.copy(_v(vv, pi * 96, [[1, 96]]), qp[:, 192:288])
        pu_cm.__exit__(None, None, None)
        # ======== phase F: FFT + LN + v ========
        pfft_cm = tc.tile_pool(name="pfft", bufs=1)
        fpool = pfft_cm.__enter__()
        qhRe = fpool.tile([40, 32, 192], BF16)
        qhIm = fpool.tile([40, 32, 192], BF16)
        for ch in range(16):
            rhs = _v(qk, ch * 2 * 192, [[1, 384]])
            if ch % 2 == 0:
                psR = psum.tile([128, 512], F32, tag="ps")
                nc.tensor.matmul(psR[:40, :384], tfRe[:], rhs, start=True,
                                 stop=True)
                nc.scalar.copy(_v(qhRe, ch * 2 * 192, [[192, 2], [1, 192]]),
                               psR[:40, :384])
                psI = psum.tile([128, 512], F32, tag="ps")
                nc.tensor.matmul(psI[:40, :384], tfIm[:], rhs, start=True,
                                 stop=True)
                nc.scalar.copy(_v(qhIm, ch * 2 * 192, [[192, 2], [1, 192]]),
                               psI[:40, :384])
            else:
                psR = dpsum.tile([128, 512], F32, tag="psd", name="psd")
                nc.tensor.matmul(psR[:40, :384], tfRe[:], rhs, start=True,
                                 stop=True)
                nc.vector.tensor_copy(
                    _v(qhRe, ch * 2 * 192, [[192, 2], [1, 192]]),
                    psR[:40, :384])
                psI = dpsum.tile([128, 512], F32, tag="psd", name="psd")
                nc.tensor.matmul(psI[:40, :384], tfIm[:], rhs, start=True,
                                 stop=True)
                nc.vector.tensor_copy(
                    _v(qhIm, ch * 2 * 192, [[192, 2], [1, 192]]),
                    psI[:40, :384])
        ohRe = fpool.tile([40, 32, 96], BF16)
        ohIm = fpool.tile([40, 32, 96], BF16)
        for hf in range(2):
            po = hf * 16
            t1 = fpool.tile([40, 16, 96], BF16, tag="pt1", name="pt1", bufs=2)
            t2 = fpool.tile([40, 16, 96], BF16, tag="pt2", name="pt2", bufs=2)
            ar = _v(qhRe, po * 192, [[192, 16], [1, 96]])
            br = _v(qhRe, po * 192 + 96, [[192, 16], [1, 96]])
            ai = _v(qhIm, po * 192, [[192, 16], [1, 96]])
            bi = _v(qhIm, po * 192 + 96, [[192, 16], [1, 96]])
            oR = ohRe[:, po:po + 16]
            oI = ohIm[:, po:po + 16]
            nc.vector.tensor_tensor(out=t1[:], in0=ar, in1=br, op=ALU.mult)
            nc.vector.tensor_tensor(out=t2[:], in0=ai, in1=bi, op=ALU.mult)
            nc.vector.tensor_tensor(out=oR, in0=t1[:], in1=t2[:],
                                    op=ALU.subtract)
            nc.vector.tensor_tensor(out=t1[:], in0=ar, in1=bi, op=ALU.mult)
            nc.vector.tensor_tensor(out=t2[:], in0=ai, in1=br, op=ALU.mult)
            nc.vector.tensor_tensor(out=oI, in0=t1[:], in1=t2[:], op=ALU.add)
        osb = fpool.tile([64, 32, 96], F32)
        for ch in range(8):
            ps = psum.tile([128, 512], F32, tag="ps")
            rhsR = _v(ohRe, ch * 4 * 96, [[1, 384]])
            rhsI = _v(ohIm, ch * 4 * 96, [[1, 384]])
            nc.tensor.matmul(ps[:64, :384], tiRe[:], rhsR, start=True,
                             stop=False)
            nc.tensor.matmul(ps[:64, :384], tiIm[:], rhsI, start=False,
                             stop=True)
            nc.scalar.copy(_v(osb, ch * 4 * 96, [[96, 4], [1, 96]]),
                           ps[:64, :384])
        # LayerNorm + v (two patch-halves for overlap with FFT)
        vo = fpool.tile([64, 32, 96], F32)
        ssum = fpool.tile([64, 32], F32)
        sq = fpool.tile([64, 32, 96], F32)
        sqs = fpool.tile([64, 32], F32)
        mu = fpool.tile([64, 32], F32)
        var = fpool.tile([64, 32], F32)
        inv = fpool.tile([64, 32], F32)
        musq = fpool.tile([64, 32], F32)
        std = fpool.tile([64, 32], F32)
        on = fpool.tile([64, 32, 96], F32)
        for hf in range(2):
            po = hf * 16
            sl = slice(po, po + 16)
            nc.vector.tensor_reduce(out=ssum[:, sl], in_=osb[:, sl],
                                    axis=AX.X, op=ALU.add)
            nc.scalar.activation(sq[:, sl], osb[:, sl], ACTF.Square)
            nc.vector.tensor_reduce(out=sqs[:, sl], in_=sq[:, sl],
                                    axis=AX.X, op=ALU.add)
            nc.scalar.activation(mu[:, sl], ssum[:, sl], ACTF.Copy,
                                 scale=1.0 / 96)
            nc.scalar.activation(var[:, sl], sqs[:, sl], ACTF.Copy,
                                 scale=1.0 / 96)
            nc.scalar.activation(musq[:, sl], mu[:, sl], ACTF.Square)
            nc.vector.tensor_tensor(out=var[:, sl], in0=var[:, sl],
                                    in1=musq[:, sl], op=ALU.subtract)
            nc.vector.tensor_scalar_add(out=var[:, sl], in0=var[:, sl],
                                        scalar1=1e-5)
            nc.scalar.activation(std[:, sl], var[:, sl], ACTF.Sqrt)
            nc.vector.reciprocal(inv[:, sl], std[:, sl])
            nc.vector.tensor_tensor(out=on[:, sl], in0=osb[:, sl],
                                    in1=_v(mu, po, [[1, 16], [0, 96]]),
                                    op=ALU.subtract)
            nc.vector.tensor_tensor(out=on[:, sl], in0=on[:, sl],
                                    in1=_v(inv, po, [[1, 16], [0, 96]]),
                                    op=ALU.mult)
            if not fold_ln:
                nc.vector.tensor_tensor(out=on[:, sl], in0=on[:, sl],
                                        in1=_v(lnw, 0, [[0, 16], [1, 96]]),
                                        op=ALU.mult)
                nc.vector.tensor_tensor(out=on[:, sl], in0=on[:, sl],
                                        in1=_v(lnb, 0, [[0, 16], [1, 96]]),
                                        op=ALU.add)
            nc.vector.tensor_tensor(out=vo[:, sl], in0=on[:, sl],
                                    in1=_v(vv, po * 96, [[96, 16], [1, 96]]),
                                    op=ALU.mult)
        voT = fpool.tile([96, BND, 128], F32)
        for pi in range(32):
            prow, pcol = divmod(pi, 16)
            ps = psum.tile([128, 512], F32, tag="ps")
            nc.tensor.transpose(ps[:96, :64],
                                _v(vo, pi * 96, [[1, 96]]),
                                ident[:64, :64])
            nc.scalar.copy(_v(voT, prow * 8 * 128 + pcol * 8,
                              [[128, 8], [1, 8]]),
                           ps[:96, :64])
        nc.sync.dma_start(vo_out[:], voT[:])
        pfft_cm.__exit__(None, None, None)
        pqkv_cm.__exit__(None, None, None)
    nc.compile()
    return nc


def build_kernel_b():
    nc = bacc.Bacc("TRN2", target_bir_lowering=False, debug=False)
    voh_d = nc.dram_tensor("voh", [96, 18, 130], F32R, kind="ExternalInput")
    ident_d = nc.dram_tensor("ident", [128, 128], F32, kind="ExternalInput")
    identr_d = nc.dram_tensor("identr", [128, 128], F32R, kind="ExternalInput")
    ow3T_d = nc.dram_tensor("ow3T", [96, 2], F32R, kind="ExternalInput")
    w3K_d = nc.dram_tensor("w3K", [96, 48], F32R, kind="ExternalInput")
    b3c_d = nc.dram_tensor("b3c", [48, 1], F32, kind="ExternalInput")
    tau3_d = nc.dram_tensor("tau3", [128, 3], F32, kind="ExternalInput")
    out_d = nc.dram_tensor("outp", [48, BND, 128], F32, kind="ExternalOutput")
    NYV = 18
    VXP = 130
    with tile.TileContext(nc) as tc, ExitStack() as top:
        pool = top.enter_context(tc.tile_pool(name="b", bufs=1))
        tpool = top.enter_context(tc.tile_pool(name="bt", bufs=2))
        psum = top.enter_context(tc.tile_pool(name="psumb", bufs=4, space="PSUM"))
        voh = pool.tile([96, NYV, VXP], F32R)
        ident = pool.tile([128, 128], F32)
        identr = pool.tile([128, 128], F32R)
        ow3T = pool.tile([96, 2], F32R)
        w3K = pool.tile([96, 48], F32R)
        b3c = pool.tile([48, 1], F32)
        tau3 = pool.tile([128, 3], F32)
        for sb, dr in [(ident, ident_d), (identr, identr_d),
                       (ow3T, ow3T_d), (w3K, w3K_d), (b3c, b3c_d),
                       (tau3, tau3_d)]:
            nc.sync.dma_start(sb[:], dr[:])
        for rc in range(3):
            nc.sync.dma_start(voh[:, 6 * rc:6 * (rc + 1)],
                              voh_d[:, 6 * rc:6 * (rc + 1)])
        vx = {t: pool.tile([128, 96, NYV], BF16, tag=f"vx{t}", name=f"vx{t}") for t in (-1, 0, 1)}
        for y in range(NYV):
            ps = psum.tile([128, 512], F32, tag="ps")
            nc.tensor.transpose(ps[:128, :96].bitcast(F32R),
                                _v(voh, y * VXP + 1, [[1, 128]]),
                                identr[:96, :96])
            nc.scalar.copy(_v(vx[0], y, [[NYV, 96]]), ps[:128, :96])
        for t in (-1, 1):
            nc.gpsimd.memset(vx[t][:], 0.0)
            if t > 0:
                nc.sync.dma_start(vx[t][0:127], vx[0][1:128])
            else:
                nc.sync.dma_start(vx[t][1:128], vx[0][0:127])
        off3pm = pool.tile([128, BND, 2], F32)
        for b in range(BND):
            ps = psum.tile([128, 512], F32, tag="ps")
            nc.tensor.matmul(ps[:128, :2],
                             _v(voh, (b + 1) * VXP + 1, [[1, 128]]),
                             ow3T[:], start=True, stop=True)
            nc.scalar.copy(_v(off3pm, b * 2, [[1, 2]]), ps[:128, :2])
        h3y = pool.tile([128, 3, BND], F32)
        h3x = pool.tile([128, 3, BND], F32)
        for hh, off in ((h3y, 0), (h3x, 1)):
            t0 = tpool.tile([128, 3, BND], F32, tag="h3tmp")
            nc.vector.tensor_tensor(
                out=t0[:], in0=_v(off3pm, off, [[0, 3], [2, BND]]),
                in1=_v(tau3, 0, [[1, 3], [0, BND]]), op=ALU.subtract)
            nc.scalar.activation(t0[:], t0[:], ACTF.Abs)
            nc.scalar.activation(hh[:], t0[:], ACTF.Relu, bias=1.0, scale=-1.0)
        m3 = pool.tile([128, 3, 3, BND], BF16)
        for t in range(3):
            nc.vector.tensor_tensor(
                out=m3[:, t], in0=_v(h3y, 0, [[BND, 3], [1, BND]]),
                in1=_v(h3x, t * BND, [[0, 3], [1, BND]]), op=ALU.mult)
        s3 = pool.tile([128, 96, BND], F32)
        for i, t in enumerate((0, -1, 1)):
            tmp = tpool.tile([128, 3, 96, BND], BF16, tag="s3tmp", bufs=2)
            nc.vector.tensor_tensor(
                out=tmp[:],
                in0=_v(vx[t], 0, [[1, 3], [NYV, 96], [1, BND]]),
                in1=_v(m3, (t + 1) * (3 * BND), [[BND, 3], [0, 96], [1, BND]]),
                op=ALU.mult)
            a3 = tpool.tile([128, 96, BND], BF16, tag="a3", bufs=2)
            with nc.allow_low_precision(reason="3-tap partial; fp32 accum"):
                nc.vector.tensor_tensor(
                    out=a3[:], in0=_v(tmp, 0, [[1, 96 * BND]]),
                    in1=_v(tmp, 96 * BND, [[1, 96 * BND]]), op=ALU.add)
                nc.vector.tensor_tensor(
                    out=a3[:], in0=a3[:],
                    in1=_v(tmp, 2 * 96 * BND, [[1, 96 * BND]]), op=ALU.add)
            if i == 0:
                nc.gpsimd.tensor_copy(s3[:], a3[:])
            else:
                nc.gpsimd.tensor_tensor(out=s3[:], in0=s3[:], in1=a3[:],
                                        op=ALU.add)
        s3T = pool.tile([96, BND, 128], F32R)
        for y in range(BND):
            ps = psum.tile([128, 512], F32, tag="ps")
            nc.tensor.transpose(ps[:96, :128],
                                _v(s3, y, [[BND, 96]]),
                                ident[:, :])
            nc.scalar.copy(_v(s3T, y * 128, [[1, 128]]), ps[:96, :128])
        outsb = pool.tile([48, BND * 128], F32)
        for pc in range(4):
            ps = psum.tile([128, 512], F32, tag="ps")
            nc.tensor.matmul(ps[:48, :512], w3K[:],
                             _v(s3T, pc * 512, [[1, 512]]),
                             start=True, stop=True)
            nc.scalar.activation(outsb[:, pc * 512:(pc + 1) * 512],
                                 ps[:48, :512], ACTF.Identity,
                                 bias=b3c[:, :], scale=1.0)
        nc.sync.dma_start(out_d[:], outsb[:])
    nc.compile()
    return nc


def _hat_consts(inputs):
    fold_ln = not np.any(inputs['ln_b'])
    lnscale = (inputs['ln_w'].astype(np.float32) if fold_ln
               else np.ones(96, np.float32))
    w1 = inputs['w1'][:, :, 0, 0].astype(np.float32)
    b1 = inputs['b1'].astype(np.float32)
    ow2 = inputs['off_w2'].astype(np.float32)
    w2f = inputs['w2'][:, 0].reshape(288, 9).astype(np.float32)
    OW = np.zeros((18, 9, 49), np.float32)
    for ty in range(3):
        for tx in range(3):
            t = ty * 3 + tx
            OW[:, t, :48] = ow2[:, :, ty, tx] @ w1
            OW[:, t, 48] = ow2[:, :, ty, tx] @ b1
    A = np.zeros((288, 9, 49), np.float32)
    for k in range(9):
        A[:, k, :48] = w2f[:, k:k + 1] * w1
        A[:, k, 48] = w2f[:, k] * b1
    Afull = np.zeros((442, 288), np.float32)
    Afull[:441] = A.reshape(288, 441).T
    Afull[441] = inputs['b2'].astype(np.float32)
    a4 = np.zeros((128, 4, 288), np.float32)
    for c in range(4):
        a4[:, c, :] = Afull[CB[c]:CB[c] + 128]
    a4[:70, 3, :] = 0.0  # chunk-3 rows 314..383 already counted in chunk 2
    Tf = np.zeros((80, 64), np.float32)
    basis = np.zeros((8, 8))
    for y in range(8):
        for x in range(8):
            basis[:] = 0.0
            basis[y, x] = 1.0
            Fz = np.fft.rfft2(basis)
            Tf[:40, y * 8 + x] = Fz.real.reshape(-1)
            Tf[40:, y * 8 + x] = Fz.imag.reshape(-1)
    Ti = np.zeros((64, 80), np.float32)
    for j in range(40):
        fy, fx = divmod(j, 5)
        Z = np.zeros((8, 5), complex)
        Z[fy, fx] = 1.0
        Ti[:, j] = np.fft.irfft2(Z, s=(8, 8)).reshape(-1)
        Z[fy, fx] = 1j
        Ti[:, 40 + j] = np.fft.irfft2(Z, s=(8, 8)).reshape(-1)
    owt = np.ascontiguousarray(OW.transpose(2, 1, 0))  # [49, 9, 18]
    return dict(
        fold_ln=fold_ln, a4=a4, owt=owt,
        tfRe=np.ascontiguousarray(Tf[0:40].T), tfIm=np.ascontiguousarray(Tf[40:80].T),
        tiRe=np.ascontiguousarray(Ti[:, 0:40].T).astype(_bf16),
        tiIm=np.ascontiguousarray(Ti[:, 40:80].T).astype(_bf16),
        ident=np.eye(128, dtype=np.float32),
        ow1T=np.ascontiguousarray(inputs['off_w1'][:, :, 0, 0].T.astype(np.float32)),
        tau3=np.tile(np.array([-1, 0, 1], np.float32), (128, 1)),
        tau5=np.tile(np.arange(-2, 3, dtype=np.float32), (128, 1)),
        lnw64=np.tile(inputs['ln_w'].astype(np.float32), (64, 1)),
        lnb64=np.tile(inputs['ln_b'].astype(np.float32), (64, 1)),
        ow3T=np.ascontiguousarray(
            (inputs['off_w3'][:, :, 0, 0] * lnscale[None, :]).T.astype(np.float32)),
        w3K=np.ascontiguousarray(
            (inputs['w3'][:, :, 0, 0] * lnscale[None, :]).T.astype(np.float32)),
        b3c=inputs['b3'].astype(np.float32).reshape(48, 1),
    )


def make_in_maps(inputs):
    """Host-side prep: consts + per-core input slices for kernel A."""
    C = _hat_consts(inputs)
    x = np.asarray(inputs['x'][0], np.float32)
    xp = np.zeros((48, H + 8, XP), np.float32)
    xp[:, 4:4 + H, 3:3 + W] = x
    in_a = []
    for ci in range(NCORES):
        onesp = np.zeros((128, NYS), np.float32)
        for j in range(NYS):
            if 0 <= 16 * ci - 3 + j < H:
                onesp[:, j] = 1.0
        in_a.append(dict(
            xh=np.ascontiguousarray(xp[:, 16 * ci:16 * ci + NYX, :]),
            onesp=onesp,
            ident=C['ident'], identr=C['ident'], ow1T=C['ow1T'], owt=C['owt'],
            a4=C['a4'], tfRe=C['tfRe'], tfIm=C['tfIm'], tiRe=C['tiRe'],
            tiIm=C['tiIm'], tau3=C['tau3'], tau5=C['tau5'],
            lnw64=C['lnw64'], lnb64=C['lnb64']))
    return C, in_a


def make_in_maps_b(C, vo_full):
    vop = np.zeros((96, H + 2, 130), np.float32)
    vop[:, 1:1 + H, 1:1 + W] = vo_full
    in_b = []
    for ci in range(NCORES):
        in_b.append(dict(
            voh=np.ascontiguousarray(vop[:, 16 * ci:16 * ci + 18, :]),
            ident=C['ident'], identr=C['ident'], ow3T=C['ow3T'], w3K=C['w3K'],
            b3c=C['b3c'], tau3=C['tau3']))
    return in_b


_CACHE = {}


def kernel(**inputs):
    C, in_a = make_in_maps(inputs)
    key = 'nca_fold' if C['fold_ln'] else 'nca'
    if key not in _CACHE:
        _CACHE[key] = build_kernel_a(fold_ln=C['fold_ln'])
        _CACHE.setdefault('ncb', build_kernel_b())
    nca, ncb = _CACHE[key], _CACHE['ncb']
    res_a = bass_utils.run_bass_kernel_spmd(nca, in_a, core_ids=list(range(NCORES)))
    vo_full = np.concatenate([r['vo_out'] for r in res_a.results], axis=1)
    in_b = make_in_maps_b(C, vo_full)
    res_b = bass_utils.run_bass_kernel_spmd(ncb, in_b, core_ids=list(range(NCORES)))
    out = np.concatenate([r['outp'] for r in res_b.results], axis=1)
    return out[None].astype(np.float32)

